# revision 42
# baseline (speedup 1.0000x reference)
"""Trainium2 Bass kernel for DGNN message passing (scatter-softmax GNN).

Math (reference):
    src, dst = edge_index[0], edge_index[2]
    alpha_e  = <entities[src_e], entities[dst_e]> / sqrt(256)
    attn     = scatter_softmax(alpha, dst)
    out[n]   = sum_{e: dst_e = n} attn_e * entities[src_e]

Sharding: destination nodes partitioned over 8 cores (12500 each), and
within a core assigned to 98 tiles of 128 nodes by a balanced bin-packing
(host-side) that equalizes per-(tile, src-bank) edge counts, so the edge
slot space is a uniform 1078 chunks of 128 edge slots per core (the
output rows are un-permuted on the host). A single bf16 row gather per
edge keeps the SWDGE descriptor count (the serial Pool-engine cost that
dominates this kernel) at one descriptor per edge slot.

Per-core pipeline (bf16 data path, fp32 accumulation):
  - qv rows (entities[src]) gathered with dma_gather from a bf16 copy of
    the table, 4 int16 banks, one call per (window, bank).
  - Per 4-chunk group: PE transposes qv -> qvT (PSUM, batched per bank),
    scores ap[slot, node] = qvT.T @ ntT_tile (the pretransposed local
    node table is SBUF-resident), exp on the scalar engine (|alpha| < 5
    so no max subtraction), msel = exp * (dstl == node) on the vector
    engine in bf16.
  - Per tile: one PSUM bank accumulates [weighted sum | segment sum]
    as a single accumulation group (cols 0..127 and col 128), via two
    matmuls per chunk sharing lhsT = msel.
  - out = W / (segsum + eps); eps preserves zeros for isolated nodes.
"""

import math

import ml_dtypes
import numpy as np

import concourse.bacc as bacc
import concourse.bass as bass
import concourse.mybir as mybir
from concourse.tile import TileContext
from concourse.masks import make_identity
from concourse.bass_utils import run_bass_kernel_spmd

P = 128
D = 128
HIDDEN_DIM = 128
SCALE = 1.0 / math.sqrt(D + HIDDEN_DIM)

N_CORES = 8
N_FULL = 100000
NPC = N_FULL // N_CORES  # 12500 destination nodes per core
NT = (NPC + P - 1) // P  # 98 node tiles per core
NLOC = NT * P  # 12544 padded local nodes
N_BANKS = 4
BANK = 25000  # bank rows (< 32768 so int16 indices work)
EPS = 1e-20
WIN = 8  # node tiles per gather window
G = 4  # chunks per score/transpose batch (one 512-col PSUM bank)
HWCH = 48  # max chunks per half-window (for the indicator batch op)

# Per-(tile, bank) slot capacities: rotating (384,384,384,256) /
# (384,384,256,256) patterns, 11/11/10 chunks per tile, 1045 chunks per
# core. Greedy node packing below fits every core's nodes within these
# caps (validated on the dataset).
CAPS_11 = np.array([384, 384, 384, 256], dtype=np.int64)
CAPS_10 = np.array([384, 384, 256, 256], dtype=np.int64)


def _layout(loose=False):
    """Shared compile-time chunk layout (identical across cores)."""
    caps = np.stack(
        [
            np.roll(CAPS_11 if (loose or t % 3 == 0) else CAPS_10, t % 4)
            for t in range(NT)
        ]
    )  # [NT, 4]
    nch = caps // P  # chunks per (t, b)
    # ramp-up schedule: small first windows so the PE starts early
    sizes = [2, 3, 5] + [WIN] * ((NT - 10 + WIN - 1) // WIN)
    windows = []
    t0 = 0
    for s in sizes:
        t1 = min(t0 + s, NT)
        if t1 > t0:
            windows.append((t0, t1))
        t0 = t1
    chunk_tile = []  # chunk -> tile
    chunk_base = np.zeros((NT, N_BANKS), np.int64)  # (t, b) -> first chunk
    win_chunk0 = []  # window -> first chunk
    win_bank_range = []  # window -> [(cb, gn)] * 4
    ci = 0
    for (t0, t1) in windows:
        win_chunk0.append(ci)
        brs = []
        for b in range(N_BANKS):
            cb = ci
            for t in range(t0, t1):
                chunk_base[t, b] = ci
                chunk_tile.extend([t] * int(nch[t, b]))
                ci += int(nch[t, b])
            brs.append((cb, ci - cb))
        win_bank_range.append(brs)
    tc = ci
    return dict(
        caps=caps, nch=nch, windows=windows,
        chunk_tile=np.array(chunk_tile), chunk_base=chunk_base,
        win_chunk0=win_chunk0, win_bank_range=win_bank_range, tc=tc,
    )


def _pack_core(deg, caps):
    """Greedy assignment of 12500 nodes to 98 tiles of <=128 nodes,
    respecting per-(tile, bank) capacities. deg: [NPC, 4] bank degrees."""
    order = np.argsort(-deg.sum(1), kind="stable")
    rem = caps.astype(np.float64).copy()
    nodes_left = np.full(NT, P, np.float64)
    tile_of = np.full(NPC, -1, np.int64)
    pos_of = np.full(NPC, -1, np.int64)
    fill = np.zeros(NT, np.int64)
    for n in order:
        d = deg[n]
        ok = (nodes_left > 0) & (rem >= d).all(1)
        assert ok.any(), "node packing failed; loosen BASE_CAPS"
        slack = (rem - d).min(1) + 0.02 * nodes_left
        slack[~ok] = -1e18
        t = int(np.argmax(slack))
        tile_of[n] = t
        pos_of[n] = fill[t]
        fill[t] += 1
        rem[t] -= d
        nodes_left[t] -= 1
    return tile_of, pos_of


def _prep_shards(entities_bf16, src, dst, lay):
    """Per-core index/table arrays for the slot layout in `lay`."""
    core = dst // NPC
    bank = src // BANK
    tc = lay["tc"]
    caps, chunk_base = lay["caps"], lay["chunk_base"]
    group_base_slot = chunk_base * P  # [NT, 4]
    shards = []
    for c in range(N_CORES):
        m = np.nonzero(core == c)[0]
        loc = (dst[m] - c * NPC).astype(np.int64)
        b = bank[m]
        deg = np.zeros((NPC, N_BANKS), np.int64)
        np.add.at(deg, (loc, b), 1)
        tile_of, pos_of = _pack_core(deg, caps)

        # permuted local node table, pretransposed: ntT[:, lrow] = row
        lrow = tile_of * P + pos_of  # node local id -> table row
        ntT = np.zeros((D, NLOC), np.float32)
        ntT[:, lrow] = entities_bf16[c * NPC : (c + 1) * NPC].astype(np.float32).T

        # edge slots: group edges by (tile(dst), bank(src))
        et = tile_of[loc]
        key = et * N_BANKS + b
        order = np.argsort(key, kind="stable")
        key_s = key[order]
        cnt = np.bincount(key, minlength=NT * N_BANKS)
        assert (cnt <= caps.ravel()).all()
        starts = np.zeros(NT * N_BANKS, np.int64)
        np.cumsum(cnt[:-1], out=starts[1:])
        offs = np.arange(len(order)) - starts[key_s]
        slot = group_base_slot.ravel()[key_s] + offs

        p_in = slot % P
        chunk = slot // P
        col = chunk * 8 + p_in // 16
        row = p_in % 16

        qidx_qv = np.zeros((16, tc * 8), np.int16)
        dstl = np.full((P, tc), -1.0, np.float32)
        es, el = src[m][order], loc[order]
        qidx_qv[row, col] = (es - b[order] * BANK).astype(np.int16)
        dstl[p_in, chunk] = (lrow[el] % P).astype(np.float32)

        shards.append(
            {
                "ntT": ntT.astype(ml_dtypes.bfloat16),
                "qidx_qv": np.tile(qidx_qv, (8, 1)),
                "dstl": dstl.astype(ml_dtypes.bfloat16),
                "lrow": lrow,  # host-side only (output unpermute)
            }
        )
    return shards


def build_program(lay):
    tc_total = lay["tc"]
    nch = lay["nch"]
    windows = lay["windows"]
    chunk_tile = lay["chunk_tile"]
    lay_chunk_base = lay["chunk_base"]
    win_chunk0 = lay["win_chunk0"]
    win_bank_range = lay["win_bank_range"]

    nc = bacc.Bacc(None, target_bir_lowering=False, num_swdge_queues=4)
    ent = nc.dram_tensor(
        "ent", [N_FULL, D], mybir.dt.bfloat16, kind="ExternalInput"
    )
    ntT = nc.dram_tensor("ntT", [P, NLOC], mybir.dt.bfloat16, kind="ExternalInput")
    qidx_qv = nc.dram_tensor(
        "qidx_qv", [P, tc_total * 8], mybir.dt.int16, kind="ExternalInput"
    )
    dstl = nc.dram_tensor(
        "dstl", [P, tc_total], mybir.dt.bfloat16, kind="ExternalInput"
    )
    out = nc.dram_tensor("out", [NLOC, D], mybir.dt.float32, kind="ExternalOutput")

    qn = 0
    with TileContext(nc) as tc:
        with (
            tc.tile_pool(name="const_pool", bufs=1) as cpool,
            tc.tile_pool(name="idx_pool", bufs=1) as ipool,
            tc.tile_pool(name="qv_pool", bufs=3) as gpool,
            tc.tile_pool(name="qvt_pool", bufs=3) as qtpool,
            tc.tile_pool(name="ind_pool", bufs=2) as indpool,
            tc.tile_pool(name="msel_pool", bufs=2) as mpool,
            tc.tile_pool(name="work_pool", bufs=4) as wpool,
            tc.tile_pool(name="out_pool", bufs=3) as opool,
            tc.tile_pool(name="tp_pool", bufs=2, space="PSUM") as tppsum,
            tc.tile_pool(name="ap_pool", bufs=2, space="PSUM") as appsum,
            tc.tile_pool(name="acc_pool", bufs=4, space="PSUM") as qpsum,
        ):
            identity = cpool.tile([P, P], mybir.dt.bfloat16)
            make_identity(nc, identity[:])
            iota_i = cpool.tile([P, P], mybir.dt.int32)
            nc.gpsimd.iota(iota_i[:], pattern=[[1, P]], base=0, channel_multiplier=0)
            iota_f = cpool.tile([P, P], mybir.dt.bfloat16)
            nc.vector.tensor_copy(iota_f[:], iota_i[:])
            ones = cpool.tile([P, 1], mybir.dt.bfloat16)
            nc.vector.memset(ones[:], 1.0)

            # input staging tiles, loaded in per-window slices inside the
            # loop (subtile deps let window 0 start without waiting for
            # the full 7.6MB of index/table data)
            ntT_sb = ipool.tile([P, NLOC], mybir.dt.bfloat16)
            dstl_sb = ipool.tile([P, tc_total], mybir.dt.bfloat16)
            qv_idx_sb = ipool.tile([P, tc_total * 8], mybir.dt.int16)

            def _emit_agg(job):
                # one PSUM bank per tile: weighted sums in cols 0..127, the
                # segment sum in col 128 — a single accumulation group
                # (start=True pending-zeroes the whole 2KB zero region).
                (t0, t1, wc0, qv, mselw) = job
                for t in range(t0, t1):
                    acc = qpsum.tile([P, 512], mybir.dt.float32, tag="acc", name="acc")
                    cs = [
                        c
                        for b in range(N_BANKS)
                        for c in range(
                            int(lay_chunk_base[t, b]),
                            int(lay_chunk_base[t, b]) + int(nch[t, b]),
                        )
                    ]
                    for i, c in enumerate(cs):
                        j = c - wc0
                        nc.tensor.matmul(
                            acc[:, 0:P],
                            lhsT=mselw[:, j, :],
                            rhs=qv[:, j, :],
                            start=(i == 0),
                            stop=False,
                        )
                        nc.tensor.matmul(
                            acc[:, P : P + 1],
                            lhsT=mselw[:, j, :],
                            rhs=ones[:],
                            start=False,
                            stop=(i == len(cs) - 1),
                        )
                    denom = wpool.tile([P, 1], mybir.dt.float32, tag="den", name="den")
                    nc.vector.tensor_scalar_add(denom[:], acc[:, P : P + 1], EPS)
                    recip = wpool.tile([P, 1], mybir.dt.float32, tag="rec", name="rec")
                    nc.vector.reciprocal(recip[:], denom[:])
                    ot = opool.tile([P, D], mybir.dt.float32, tag="ot", name="ot")
                    nc.scalar.activation(
                        ot[:],
                        acc[:, 0:P],
                        mybir.ActivationFunctionType.Copy,
                        scale=recip[:],
                    )
                    nc.sync.dma_start(out=out[t * P : (t + 1) * P, :], in_=ot[:])

            pend = []
            for w, (t0, t1) in enumerate(windows):
                wc0 = win_chunk0[w]
                wch = int(nch[t0:t1].sum())

                nc.sync.dma_start(
                    out=qv_idx_sb[:, wc0 * 8 : (wc0 + wch) * 8],
                    in_=qidx_qv[:, wc0 * 8 : (wc0 + wch) * 8],
                )
                nc.sync.dma_start(
                    out=ntT_sb[:, t0 * P : t1 * P], in_=ntT[:, t0 * P : t1 * P]
                )
                nc.sync.dma_start(
                    out=dstl_sb[:, wc0 : wc0 + wch], in_=dstl[:, wc0 : wc0 + wch]
                )
                qv = gpool.tile([P, wch, D], mybir.dt.bfloat16, tag="qv", name="qv")
                for b in range(N_BANKS):
                    cb, gn = win_bank_range[w][b]
                    ni = gn * P
                    nc.gpsimd.dma_gather(
                        qv[:, cb - wc0 : cb - wc0 + gn, :],
                        ent[b * BANK : (b + 1) * BANK, :],
                        qv_idx_sb[:, cb * 8 : (cb + gn) * 8],
                        ni,
                        ni,
                        D,
                        single_packet=False,
                        queue_num=qn % 4,
                    )
                    qn += 1

                # previous window's aggregation first: the PE starts each
                # window with guaranteed-ready matmuls (mselw/qv of w-1)
                # instead of stalling on this window's gather drain
                if pend:
                    _emit_agg(pend.pop(0))

                mselw = mpool.tile(
                    [P, wch, P], mybir.dt.bfloat16, tag="mselw", name="mselw"
                )

                def _emit_scores(job, wc0=wc0, mselw=mselw):
                    # scores ap[slot, node] per chunk against its tile,
                    # then exp straight into the msel window buffer (the
                    # indicator mask is applied in place per half-window).
                    # Runs one group behind the transposes so the PE never
                    # stalls on the PSUM->SBUF qvT copy.
                    (g0, gs, qvT) = job
                    ap = appsum.tile([P, 512], mybir.dt.float32, tag="ap", name="ap")
                    for j in range(gs):
                        t = int(chunk_tile[wc0 + g0 + j])
                        nc.tensor.matmul(
                            ap[:, j * P : (j + 1) * P],
                            lhsT=qvT[:, j * P : (j + 1) * P],
                            rhs=ntT_sb[:, t * P : (t + 1) * P],
                            start=True,
                            stop=True,
                        )
                    nc.scalar.activation(
                        mselw[:, g0 : g0 + gs, :].rearrange("p g n -> p (g n)"),
                        ap[:, : gs * P],
                        mybir.ActivationFunctionType.Exp,
                        scale=SCALE,
                    )

                gjobs = []
                for gi, g0 in enumerate(range(0, wch, G)):
                    gs = min(G, wch - g0)
                    # qv -> qvT via PE (batched into one bf16 PSUM bank)
                    tp = tppsum.tile([P, 512], mybir.dt.bfloat16, tag="tp", name="tp")
                    for j in range(gs):
                        nc.tensor.transpose(
                            tp[:, j * P : (j + 1) * P],
                            qv[:, g0 + j, :],
                            identity[:],
                        )
                    qvT = qtpool.tile([P, G * P], mybir.dt.bfloat16, tag="qvT", name="qvT")
                    if gi % 5 < 3:
                        nc.vector.tensor_copy(qvT[:, : gs * P], tp[:, : gs * P])
                    else:
                        nc.scalar.copy(qvT[:, : gs * P], tp[:, : gs * P])
                    gjobs.append((g0, gs, qvT))
                    if len(gjobs) == 2:
                        _emit_scores(gjobs.pop(0))
                _emit_scores(gjobs.pop(0))
                # mselw[slot, c, n] *= (dstl[slot, c] == n), two big ops
                ng = (wch + G - 1) // G
                h0 = (ng // 2) * G
                for lo, hi in ((0, h0), (h0, wch)):
                    hs = hi - lo
                    ind = indpool.tile(
                        [P, HWCH, P], mybir.dt.bfloat16, tag="ind", name="ind"
                    )
                    nc.vector.tensor_tensor(
                        out=ind[:, :hs, :],
                        in0=iota_f[:, None, :].to_broadcast([P, hs, P]),
                        in1=dstl_sb[:, wc0 + lo : wc0 + hi, None].to_broadcast(
                            [P, hs, P]
                        ),
                        op=mybir.AluOpType.is_equal,
                    )
                    nc.vector.tensor_tensor(
                        out=mselw[:, lo:hi, :],
                        in0=mselw[:, lo:hi, :],
                        in1=ind[:, :hs, :],
                        op=mybir.AluOpType.mult,
                    )

                # aggregation pass runs one window behind (software pipeline:
                # the PE never waits on this window's mask ops)
                pend.append((t0, t1, wc0, qv, mselw))
            _emit_agg(pend.pop(0))
    nc.compile()
    return nc


def kernel(entities, relations, edge_index, _trace=False):
    entities = np.ascontiguousarray(entities, dtype=np.float32)
    src = np.asarray(edge_index[0], dtype=np.int64)
    dst = np.asarray(edge_index[2], dtype=np.int64)
    assert entities.shape == (N_FULL, D)

    ent_bf16 = entities.astype(ml_dtypes.bfloat16)
    try:
        lay = _layout()
        shards = _prep_shards(ent_bf16, src, dst, lay)
    except AssertionError:
        # packing fallback for edge distributions the tight caps can't fit
        lay = _layout(loose=True)
        shards = _prep_shards(ent_bf16, src, dst, lay)
    nc = build_program(lay)

    in_maps = []
    for c in range(N_CORES):
        in_maps.append(
            {
                "ent": ent_bf16,
                "ntT": shards[c]["ntT"],
                "qidx_qv": shards[c]["qidx_qv"],
                "dstl": shards[c]["dstl"],
            }
        )
    res = run_bass_kernel_spmd(
        nc, in_maps, core_ids=list(range(N_CORES)), trace=_trace
    )
    full = np.empty((N_FULL, D), np.float32)
    for c in range(N_CORES):
        full[c * NPC : (c + 1) * NPC] = res.results[c]["out"][shards[c]["lrow"]]
    if _trace:
        kernel.last_results = res
    return full


# revision 44
# speedup vs baseline: 1.1589x; 1.1589x over previous
"""Trainium2 Bass kernel for DGNN message passing (scatter-softmax GNN).

Math (reference):
    src, dst = edge_index[0], edge_index[2]
    alpha_e  = <entities[src_e], entities[dst_e]> / sqrt(256)
    attn     = scatter_softmax(alpha, dst)
    out[n]   = sum_{e: dst_e = n} attn_e * entities[src_e]

Sharding: destination nodes partitioned over 8 cores (12500 each), and
within a core assigned to 98 tiles of 128 nodes by a balanced bin-packing
(host-side) that equalizes per-(tile, src-bank) edge counts, so the edge
slot space is a uniform 1078 chunks of 128 edge slots per core (the
output rows are un-permuted on the host). A single bf16 row gather per
edge keeps the SWDGE descriptor count (the serial Pool-engine cost that
dominates this kernel) at one descriptor per edge slot.

Per-core pipeline (bf16 data path, fp32 accumulation):
  - qv rows (entities[src]) gathered with dma_gather from a bf16 copy of
    the table, 4 int16 banks, one call per (window, bank).
  - Per 4-chunk group: PE transposes qv -> qvT (PSUM, batched per bank),
    scores ap[slot, node] = qvT.T @ ntT_tile (the pretransposed local
    node table is SBUF-resident), exp on the scalar engine (|alpha| < 5
    so no max subtraction), msel = exp * (dstl == node) on the vector
    engine in bf16.
  - Per tile: one PSUM bank accumulates [weighted sum | segment sum]
    as a single accumulation group (cols 0..127 and col 128), via two
    matmuls per chunk sharing lhsT = msel.
  - out = W / (segsum + eps); eps preserves zeros for isolated nodes.
"""

import math

import ml_dtypes
import numpy as np

import concourse.bacc as bacc
import concourse.bass as bass
import concourse.mybir as mybir
from concourse.tile import TileContext
from concourse.masks import make_identity
from concourse.bass_utils import run_bass_kernel_spmd

P = 128
D = 128
HIDDEN_DIM = 128
SCALE = 1.0 / math.sqrt(D + HIDDEN_DIM)

N_CORES = 8
N_FULL = 100000
NPC = N_FULL // N_CORES  # 12500 destination nodes per core
NT = (NPC + P - 1) // P  # 98 node tiles per core
NLOC = NT * P  # 12544 padded local nodes
N_BANKS = 4
BANK = 25000  # bank rows (< 32768 so int16 indices work)
EPS = 1e-20
WIN = 8  # node tiles per gather window
G = 4  # chunks per score/transpose batch (one 512-col PSUM bank)
HWCH = 48  # max chunks per half-window (for the indicator batch op)

# Per-(tile, bank) slot capacities: rotating (384,384,384,256) /
# (384,384,256,256) patterns, 11/11/10 chunks per tile, 1045 chunks per
# core. Greedy node packing below fits every core's nodes within these
# caps (validated on the dataset).
CAPS_11 = np.array([384, 384, 384, 256], dtype=np.int64)
CAPS_10 = np.array([384, 384, 256, 256], dtype=np.int64)


def _layout(loose=False):
    """Shared compile-time chunk layout (identical across cores)."""
    caps = np.stack(
        [
            np.roll(CAPS_11 if (loose or t % 3 == 0) else CAPS_10, t % 4)
            for t in range(NT)
        ]
    )  # [NT, 4]
    nch = caps // P  # chunks per (t, b)
    # ramp-up schedule: small first windows so the PE starts early
    sizes = [2, 3, 5] + [WIN] * ((NT - 10 + WIN - 1) // WIN)
    windows = []
    t0 = 0
    for s in sizes:
        t1 = min(t0 + s, NT)
        if t1 > t0:
            windows.append((t0, t1))
        t0 = t1
    chunk_tile = []  # chunk -> tile
    chunk_base = np.zeros((NT, N_BANKS), np.int64)  # (t, b) -> first chunk
    win_chunk0 = []  # window -> first chunk
    win_bank_range = []  # window -> [(cb, gn)] * 4
    ci = 0
    for (t0, t1) in windows:
        win_chunk0.append(ci)
        brs = []
        for b in range(N_BANKS):
            cb = ci
            for t in range(t0, t1):
                chunk_base[t, b] = ci
                chunk_tile.extend([t] * int(nch[t, b]))
                ci += int(nch[t, b])
            brs.append((cb, ci - cb))
        win_bank_range.append(brs)
    tc = ci
    return dict(
        caps=caps, nch=nch, windows=windows,
        chunk_tile=np.array(chunk_tile), chunk_base=chunk_base,
        win_chunk0=win_chunk0, win_bank_range=win_bank_range, tc=tc,
    )


def _pack_core(deg, caps):
    """Greedy assignment of 12500 nodes to 98 tiles of <=128 nodes,
    respecting per-(tile, bank) capacities. deg: [NPC, 4] bank degrees."""
    order = np.argsort(-deg.sum(1), kind="stable")
    rem = caps.astype(np.float64).copy()
    nodes_left = np.full(NT, P, np.float64)
    tile_of = np.full(NPC, -1, np.int64)
    pos_of = np.full(NPC, -1, np.int64)
    fill = np.zeros(NT, np.int64)
    for n in order:
        d = deg[n]
        ok = (nodes_left > 0) & (rem >= d).all(1)
        assert ok.any(), "node packing failed; loosen BASE_CAPS"
        slack = (rem - d).min(1) + 0.02 * nodes_left
        slack[~ok] = -1e18
        t = int(np.argmax(slack))
        tile_of[n] = t
        pos_of[n] = fill[t]
        fill[t] += 1
        rem[t] -= d
        nodes_left[t] -= 1
    return tile_of, pos_of


def _prep_shards(entities_bf16, src, dst, lay):
    """Per-core index/table arrays for the slot layout in `lay`."""
    core = dst // NPC
    bank = src // BANK
    tc = lay["tc"]
    caps, chunk_base = lay["caps"], lay["chunk_base"]
    group_base_slot = chunk_base * P  # [NT, 4]
    shards = []
    for c in range(N_CORES):
        m = np.nonzero(core == c)[0]
        loc = (dst[m] - c * NPC).astype(np.int64)
        b = bank[m]
        deg = np.zeros((NPC, N_BANKS), np.int64)
        np.add.at(deg, (loc, b), 1)
        tile_of, pos_of = _pack_core(deg, caps)

        # permuted local node table, pretransposed: ntT[:, lrow] = row
        lrow = tile_of * P + pos_of  # node local id -> table row
        ntT = np.zeros((D, NLOC), np.float32)
        ntT[:, lrow] = entities_bf16[c * NPC : (c + 1) * NPC].astype(np.float32).T

        # edge slots: group edges by (tile(dst), bank(src))
        et = tile_of[loc]
        key = et * N_BANKS + b
        order = np.argsort(key, kind="stable")
        key_s = key[order]
        cnt = np.bincount(key, minlength=NT * N_BANKS)
        assert (cnt <= caps.ravel()).all()
        starts = np.zeros(NT * N_BANKS, np.int64)
        np.cumsum(cnt[:-1], out=starts[1:])
        offs = np.arange(len(order)) - starts[key_s]
        slot = group_base_slot.ravel()[key_s] + offs

        p_in = slot % P
        chunk = slot // P
        col = chunk * 8 + p_in // 16
        row = p_in % 16

        qidx_qv = np.zeros((16, tc * 8), np.int16)
        dstl = np.full((P, tc), -1.0, np.float32)
        es, el = src[m][order], loc[order]
        qidx_qv[row, col] = (es - b[order] * BANK).astype(np.int16)
        dstl[p_in, chunk] = (lrow[el] % P).astype(np.float32)

        shards.append(
            {
                "ntT": ntT.astype(ml_dtypes.bfloat16),
                "qidx_qv": np.tile(qidx_qv, (8, 1)),
                "dstl": dstl.astype(ml_dtypes.bfloat16),
                "lrow": lrow,  # host-side only (output unpermute)
            }
        )
    return shards


def build_program(lay):
    tc_total = lay["tc"]
    nch = lay["nch"]
    windows = lay["windows"]
    chunk_tile = lay["chunk_tile"]
    lay_chunk_base = lay["chunk_base"]
    win_chunk0 = lay["win_chunk0"]
    win_bank_range = lay["win_bank_range"]

    nc = bacc.Bacc(None, target_bir_lowering=False, num_swdge_queues=4)
    ent = nc.dram_tensor(
        "ent", [N_FULL, D], mybir.dt.bfloat16, kind="ExternalInput"
    )
    ntT = nc.dram_tensor("ntT", [P, NLOC], mybir.dt.bfloat16, kind="ExternalInput")
    qidx_qv = nc.dram_tensor(
        "qidx_qv", [P, tc_total * 8], mybir.dt.int16, kind="ExternalInput"
    )
    dstl = nc.dram_tensor(
        "dstl", [P, tc_total], mybir.dt.bfloat16, kind="ExternalInput"
    )
    out = nc.dram_tensor("out", [NLOC, D], mybir.dt.float32, kind="ExternalOutput")

    qn = 0
    with TileContext(nc) as tc:
        with (
            tc.tile_pool(name="const_pool", bufs=1) as cpool,
            tc.tile_pool(name="idx_pool", bufs=1) as ipool,
            tc.tile_pool(name="qv_pool", bufs=3) as gpool,
            tc.tile_pool(name="qvt_pool", bufs=3) as qtpool,
            tc.tile_pool(name="ind_pool", bufs=2) as indpool,
            tc.tile_pool(name="msel_pool", bufs=2) as mpool,
            tc.tile_pool(name="work_pool", bufs=4) as wpool,
            tc.tile_pool(name="out_pool", bufs=3) as opool,
            tc.tile_pool(name="tp_pool", bufs=2, space="PSUM") as tppsum,
            tc.tile_pool(name="ap_pool", bufs=2, space="PSUM") as appsum,
            tc.tile_pool(name="acc_pool", bufs=4, space="PSUM") as qpsum,
        ):
            identity = cpool.tile([P, P], mybir.dt.bfloat16)
            make_identity(nc, identity[:])
            iota_i = cpool.tile([P, P], mybir.dt.int32)
            nc.gpsimd.iota(iota_i[:], pattern=[[1, P]], base=0, channel_multiplier=0)
            iota_f = cpool.tile([P, P], mybir.dt.bfloat16)
            nc.vector.tensor_copy(iota_f[:], iota_i[:])
            ones = cpool.tile([P, 1], mybir.dt.bfloat16)
            nc.vector.memset(ones[:], 1.0)

            # input staging tiles, loaded in per-window slices inside the
            # loop (subtile deps let window 0 start without waiting for
            # the full 7.6MB of index/table data)
            ntT_sb = ipool.tile([P, NLOC], mybir.dt.bfloat16)
            dstl_sb = ipool.tile([P, tc_total], mybir.dt.bfloat16)
            qv_idx_sb = ipool.tile([P, tc_total * 8], mybir.dt.int16)

            def _emit_agg(job):
                # one PSUM bank per tile: weighted sums in cols 0..127, the
                # segment sum in col 128 — a single accumulation group
                # (start=True pending-zeroes the whole 2KB zero region).
                (t0, t1, wc0, qv, mselw) = job
                for t in range(t0, t1):
                    acc = qpsum.tile([P, 512], mybir.dt.float32, tag="acc", name="acc")
                    cs = [
                        c
                        for b in range(N_BANKS)
                        for c in range(
                            int(lay_chunk_base[t, b]),
                            int(lay_chunk_base[t, b]) + int(nch[t, b]),
                        )
                    ]
                    for i, c in enumerate(cs):
                        j = c - wc0
                        nc.tensor.matmul(
                            acc[:, 0:P],
                            lhsT=mselw[:, j, :],
                            rhs=qv[:, j, :],
                            start=(i == 0),
                            stop=False,
                        )
                        nc.tensor.matmul(
                            acc[:, P : P + 1],
                            lhsT=mselw[:, j, :],
                            rhs=ones[:],
                            start=False,
                            stop=(i == len(cs) - 1),
                        )
                    denom = wpool.tile([P, 1], mybir.dt.float32, tag="den", name="den")
                    nc.vector.tensor_scalar_add(denom[:], acc[:, P : P + 1], EPS)
                    recip = wpool.tile([P, 1], mybir.dt.float32, tag="rec", name="rec")
                    nc.vector.reciprocal(recip[:], denom[:])
                    ot = opool.tile([P, D], mybir.dt.float32, tag="ot", name="ot")
                    nc.scalar.activation(
                        ot[:],
                        acc[:, 0:P],
                        mybir.ActivationFunctionType.Copy,
                        scale=recip[:],
                    )
                    nc.sync.dma_start(out=out[t * P : (t + 1) * P, :], in_=ot[:])

            pend = []
            for w, (t0, t1) in enumerate(windows):
                wc0 = win_chunk0[w]
                wch = int(nch[t0:t1].sum())

                nc.sync.dma_start(
                    out=qv_idx_sb[:, wc0 * 8 : (wc0 + wch) * 8],
                    in_=qidx_qv[:, wc0 * 8 : (wc0 + wch) * 8],
                )
                nc.sync.dma_start(
                    out=ntT_sb[:, t0 * P : t1 * P], in_=ntT[:, t0 * P : t1 * P]
                )
                nc.sync.dma_start(
                    out=dstl_sb[:, wc0 : wc0 + wch], in_=dstl[:, wc0 : wc0 + wch]
                )
                qv = gpool.tile([P, wch, D], mybir.dt.bfloat16, tag="qv", name="qv")
                for b in range(N_BANKS):
                    cb, gn = win_bank_range[w][b]
                    ni = gn * P
                    nc.gpsimd.dma_gather(
                        qv[:, cb - wc0 : cb - wc0 + gn, :],
                        ent[b * BANK : (b + 1) * BANK, :],
                        qv_idx_sb[:, cb * 8 : (cb + gn) * 8],
                        ni,
                        ni,
                        D,
                        single_packet=False,
                        queue_num=qn % 4,
                    )
                    qn += 1

                mselw = mpool.tile(
                    [P, wch, P], mybir.dt.bfloat16, tag="mselw", name="mselw"
                )

                def _emit_scores(job, wc0=wc0, mselw=mselw):
                    # scores ap[slot, node] per chunk against its tile,
                    # then exp straight into the msel window buffer (the
                    # indicator mask is applied in place per half-window).
                    # Runs one group behind the transposes so the PE never
                    # stalls on the PSUM->SBUF qvT copy.
                    (g0, gs, qvT) = job
                    ap = appsum.tile([P, 512], mybir.dt.float32, tag="ap", name="ap")
                    for j in range(gs):
                        t = int(chunk_tile[wc0 + g0 + j])
                        nc.tensor.matmul(
                            ap[:, j * P : (j + 1) * P],
                            lhsT=qvT[:, j * P : (j + 1) * P],
                            rhs=ntT_sb[:, t * P : (t + 1) * P],
                            start=True,
                            stop=True,
                        )
                    nc.scalar.activation(
                        mselw[:, g0 : g0 + gs, :].rearrange("p g n -> p (g n)"),
                        ap[:, : gs * P],
                        mybir.ActivationFunctionType.Exp,
                        scale=SCALE,
                    )

                gjobs = []
                for gi, g0 in enumerate(range(0, wch, G)):
                    gs = min(G, wch - g0)
                    # qv -> qvT via PE (batched into one bf16 PSUM bank)
                    tp = tppsum.tile([P, 512], mybir.dt.bfloat16, tag="tp", name="tp")
                    for j in range(gs):
                        nc.tensor.transpose(
                            tp[:, j * P : (j + 1) * P],
                            qv[:, g0 + j, :],
                            identity[:],
                        )
                    qvT = qtpool.tile([P, G * P], mybir.dt.bfloat16, tag="qvT", name="qvT")
                    if gi % 5 < 3:
                        nc.vector.tensor_copy(qvT[:, : gs * P], tp[:, : gs * P])
                    else:
                        nc.scalar.copy(qvT[:, : gs * P], tp[:, : gs * P])
                    gjobs.append((g0, gs, qvT))
                    if len(gjobs) == 2:
                        _emit_scores(gjobs.pop(0))
                _emit_scores(gjobs.pop(0))
                # mselw[slot, c, n] *= (dstl[slot, c] == n), two big ops
                ng = (wch + G - 1) // G
                h0 = (ng // 2) * G
                for lo, hi in ((0, h0), (h0, wch)):
                    hs = hi - lo
                    ind = indpool.tile(
                        [P, HWCH, P], mybir.dt.bfloat16, tag="ind", name="ind"
                    )
                    nc.vector.tensor_tensor(
                        out=ind[:, :hs, :],
                        in0=iota_f[:, None, :].to_broadcast([P, hs, P]),
                        in1=dstl_sb[:, wc0 + lo : wc0 + hi, None].to_broadcast(
                            [P, hs, P]
                        ),
                        op=mybir.AluOpType.is_equal,
                    )
                    nc.vector.tensor_tensor(
                        out=mselw[:, lo:hi, :],
                        in0=mselw[:, lo:hi, :],
                        in1=ind[:, :hs, :],
                        op=mybir.AluOpType.mult,
                    )

                # aggregation pass runs one window behind (software pipeline:
                # the PE never waits on this window's mask ops)
                pend.append((t0, t1, wc0, qv, mselw))
                if len(pend) == 2:
                    _emit_agg(pend.pop(0))
            _emit_agg(pend.pop(0))
    nc.compile()
    return nc


def kernel(entities, relations, edge_index, _trace=False):
    entities = np.ascontiguousarray(entities, dtype=np.float32)
    src = np.asarray(edge_index[0], dtype=np.int64)
    dst = np.asarray(edge_index[2], dtype=np.int64)
    assert entities.shape == (N_FULL, D)

    ent_bf16 = entities.astype(ml_dtypes.bfloat16)
    try:
        lay = _layout()
        shards = _prep_shards(ent_bf16, src, dst, lay)
    except AssertionError:
        # packing fallback for edge distributions the tight caps can't fit
        lay = _layout(loose=True)
        shards = _prep_shards(ent_bf16, src, dst, lay)
    nc = build_program(lay)

    in_maps = []
    for c in range(N_CORES):
        in_maps.append(
            {
                "ent": ent_bf16,
                "ntT": shards[c]["ntT"],
                "qidx_qv": shards[c]["qidx_qv"],
                "dstl": shards[c]["dstl"],
            }
        )
    res = run_bass_kernel_spmd(
        nc, in_maps, core_ids=list(range(N_CORES)), trace=_trace
    )
    full = np.empty((N_FULL, D), np.float32)
    for c in range(N_CORES):
        full[c * NPC : (c + 1) * NPC] = res.results[c]["out"][shards[c]["lrow"]]
    if _trace:
        kernel.last_results = res
    return full
